# revision 25
# baseline (speedup 1.0000x reference)
"""Block-diagonal causal GQA attention with RoPE, sharded over 8 TRN2 cores.

Problem (hardcoded from the spec):
  x [4096, 4096], wq [4096, 4096] (32 q heads x 128), wk/wv [4096, 1024]
  (8 kv heads), wo [4096, 4096], freqs_cos/sin [4096, 64], block_size 1024.
  4 independent causal blocks of 1024 tokens.

Sharding: 8 cores = 4 sequence blocks x 2 head-groups.  Core (b, g)
computes block b for q-heads [16g, 16g+16) (kv heads [4g, 4g+4)) and the
partial output projection through the matching rows of wo.  The host sums
the two head-group partials per block and concatenates the blocks.

v2 design (vs the fp32r baseline):
  - all matmul operands bf16 (hosts converts); psum stays f32.  bf16 is
    1 cyc/row at any width (fp32r degrades 4x below 256) and halves DMA.
  - single fused PE stream: 4 kv-groups, each group = [k sweep, 4 q
    sweeps] over the full resident xbT; the attention of group g-1 is
    interleaved into group g's sweeps as filler so the PE never idles
    waiting on ACT exp / DVE reciprocal chains.
  - causal mask applied POST-exp as a bf16 mask multiply (DVE) so the
    S->exp critical path has no DVE hop before ACT.
  - denominator via ones-matmul on the trapezoid expS layout; broadcast
    of the sum via K=1 matmul; reciprocal on [128,512] (full DVE lanes).
  - WO restructured: stationary oT slice held for 4 matmuls (nch quads),
    wo streamed bf16 per half-pass.
"""

import numpy as np
from contextlib import ExitStack

import concourse.bass as bass
import concourse.bass_isa as bass_isa
import concourse.tile as tile
import concourse.mybir as mybir
from concourse import bass_utils, library_config

F32 = mybir.dt.float32
BF16 = mybir.dt.bfloat16

DIM = 4096
BLOCK = 1024
D = 128            # head dim
HQ = 16            # q heads per core
HKV = 4            # kv heads per core
GROUPS = 4         # kv groups per core (rep = HQ // HKV)
N_CORES = 8
NEG = -1.0e9
W = 512            # psum bank width (f32)
NI = BLOCK // 128  # j-tiles per block (8)
KC = DIM // 128    # contraction chunks (32)
SCALE = float(1.0 / np.sqrt(D))

# bf16 1.0 pair packed as f32 bits, for memset on bf16 tiles
BF16_ONES_F32 = float(np.array([0x3F803F80], dtype=np.uint32).view(np.float32)[0])


def _trim_dma_waits(nc):
    """Drop DMA semaphore waits that are transitively guaranteed.

    The DGE descriptor path supports only 2 sync-wait commands per DMA,
    but Tile's wait emission is not transitively minimal.  We compute,
    for every instruction, a conservative "floor": the semaphore values
    guaranteed to have been reached by the time it completes (its own
    waits, the floors of the instructions those waits observe, the
    floors of its sync dependencies, plus in-order completion along each
    semaphore's single FIFO ring).  A wait on a DMA is dead if the
    floors implied by its remaining waits already cover it.
    """
    import bass_rust

    insts = []
    for blk in nc.m.functions[0].blocks:
        insts.extend(blk.instructions)

    floors: dict[str, dict[int, int]] = {}     # inst name -> {sem id: value}
    chain: dict[int, list[tuple[int, str]]] = {}  # sem id -> [(post_val, name)]
    cum: dict[int, int] = {}

    def sem_floor(sem_id, v):
        lst = chain.get(sem_id)
        if not lst:
            return None
        import bisect
        idx = bisect.bisect_left(lst, (v, ""))
        if idx == len(lst):
            return None
        return floors.get(lst[idx][1])

    def merge(dst, src):
        if not src:
            return
        for k, v in src.items():
            if dst.get(k, -1) < v:
                dst[k] = v

    for ins in insts:
        si = ins.sync_info
        fl: dict[int, int] = {}
        if si is not None:
            for w in si.on_wait:
                if w.wait_mode != "sem-ge-imm" or w.wait_value is None:
                    continue
                if fl.get(w.id, -1) < w.wait_value:
                    fl[w.id] = w.wait_value
                merge(fl, sem_floor(w.id, w.wait_value))
        try:
            for dn in ins.sync_dependency_names():
                merge(fl, floors.get(dn))
        except TypeError:
            pass
        if si is not None:
            for u in si.on_update:
                if u.update_mode not in ("sem-add-imm", "sem-inc") \
                        or u.update_value is None:
                    continue
                post = cum.get(u.id, 0) + u.update_value
                cum[u.id] = post
                lst = chain.setdefault(u.id, [])
                if lst:
                    merge(fl, floors.get(lst[-1][1]))
                if fl.get(u.id, -1) < post:
                    fl[u.id] = post
                lst.append((post, ins.name))
        floors[ins.name] = fl

    for ins in insts:
        if not isinstance(ins, mybir.InstDMACopy):
            continue
        si = ins.sync_info
        if si is None:
            continue
        waits = list(si.on_wait)
        changed = True
        while len(waits) > 1 and changed:
            changed = False
            for i, w in enumerate(waits):
                if w.wait_mode != "sem-ge-imm" or w.wait_value is None:
                    continue
                implied: dict[int, int] = {}
                for j, w2 in enumerate(waits):
                    if j == i or w2.wait_mode != "sem-ge-imm":
                        continue
                    merge(implied, sem_floor(w2.id, w2.wait_value))
                if implied.get(w.id, -1) >= w.wait_value:
                    waits.pop(i)
                    changed = True
                    break
        if len(waits) != len(si.on_wait):
            ins.sync_info = bass_rust.SyncInfo(
                on_wait=waits, on_update=list(si.on_update))


def _split_waits_json(bir):
    """Split multi-wait instructions at the BIR level.

    walrus' setupSyncWait budget: one wait of any value, or two waits
    whose values both fit a one-byte command.  Excess waits move onto
    standalone EventSemaphore instructions inserted directly before the
    instruction on the same engine.
    """
    nid = 0
    for fn in bir["functions"]:
        for blk in fn["blocks"]:
            out = []
            for ins in blk["instructions"]:
                si = ins.get("sync_info")
                waits = (si or {}).get("on_wait") or []
                if len(waits) > 1:
                    waits = sorted(
                        waits, key=lambda w: -(w.get("wait_value") or 0))
                    for w in waits[1:]:
                        nid += 1
                        out.append({
                            "debug": ins.get("debug"),
                            "engine": ins["engine"],
                            "ins": [],
                            "outs": [],
                            "name": f"{ins['name']}-w{nid}",
                            "opcode": "EventSemaphore",
                            "sync_info": {"on_update": [], "on_wait": [w]},
                        })
                    si["on_wait"] = waits[:1]
                out.append(ins)
            blk["instructions"] = out
    return bir


# expS free-dim trapezoid layout: j-tile t occupies
# [OFFS[t], OFFS[t] + BLOCK - 128 t)
OFFS = []
_o = 0
for _t in range(NI):
    OFFS.append(_o)
    _o += BLOCK - _t * 128
EW = _o  # 4608


def build_kernel():
    nc = bass.Bass("TRN2", target_bir_lowering=False, debug=False)

    xbT_d = nc.dram_tensor("xbT", [DIM, BLOCK], BF16, kind="ExternalInput").ap()
    wq_d = nc.dram_tensor("wq", [DIM, HQ * D], BF16, kind="ExternalInput").ap()
    wk_d = nc.dram_tensor("wk", [DIM, HKV * D], BF16, kind="ExternalInput").ap()
    wv_d = nc.dram_tensor("wv", [DIM, HKV * D], BF16, kind="ExternalInput").ap()
    wo_d = nc.dram_tensor("wo", [HQ * D, DIM], BF16, kind="ExternalInput").ap()
    cos_d = nc.dram_tensor("cos2", [D, BLOCK], BF16, kind="ExternalInput").ap()
    sin_d = nc.dram_tensor("sin2", [D, BLOCK], BF16, kind="ExternalInput").ap()
    out_d = nc.dram_tensor("out", [BLOCK, DIM], BF16, kind="ExternalOutput").ap()

    def mm(out_ap, lhsT, rhs, **kw):
        nc.tensor.matmul(out_ap, lhsT, rhs, **kw)

    with tile.TileContext(nc) as tc, ExitStack() as ctx:
        const = ctx.enter_context(tc.tile_pool(name="const", bufs=1))
        # softmax denominator runs on GpSimd custom ops (attn library)
        nc.gpsimd.load_library(library_config.attn)
        # bf16 causal mask for the diagonal 128x128 strip of each S^T
        # j-tile: keep (1.0) where i_local >= j_local else 0.0
        tri_f = const.tile([128, 128], F32)
        nc.gpsimd.memset(tri_f[:], 1.0)
        nc.gpsimd.affine_select(
            out=tri_f[:], in_=tri_f[:],
            compare_op=mybir.AluOpType.is_ge,
            fill=0.0, base=0, pattern=[[1, 128]], channel_multiplier=-1,
        )
        maskb = const.tile([128, 128], BF16)
        nc.scalar.copy(maskb[:], tri_f[:])

        # O^T persists through attention into the WO phase
        oT_pool = ctx.enter_context(
            tc.tile_pool(name="oTp", bufs=1, side="right"))
        oTall = oT_pool.tile([128, HQ * BLOCK], BF16, name="oTall")

        att_ps = ExitStack()      # stps/pvsp: closed manually after tail
        phase1 = ExitStack()      # x/weight pools + qk psum: closed after groups

        with tc.tile_pool(name="accs", bufs=1) as accs, \
             tc.tile_pool(name="attsb", bufs=2) as attsb:

            qTa = accs.tile([128, HQ * BLOCK], BF16, name="qTa")
            kTa = accs.tile([128, 2 * BLOCK], BF16, name="kTa")
            va = accs.tile([128, NI * HKV * D], BF16, name="va")
            cos_sb = accs.tile([D, BLOCK], BF16, name="cos_sb")
            sin_sb = accs.tile([D, BLOCK], BF16, name="sin_sb")

            xbp = phase1.enter_context(tc.tile_pool(name="xbp", bufs=1))
            wqp = phase1.enter_context(tc.tile_pool(name="wqp", bufs=32))
            wkp = phase1.enter_context(tc.tile_pool(name="wkp", bufs=1))
            wvp = phase1.enter_context(tc.tile_pool(name="wvp", bufs=11))
            ropep = phase1.enter_context(tc.tile_pool(name="ropep", bufs=1))

            xb = [xbp.tile([128, BLOCK], BF16, name=f"xb{k}", tag=f"xb{k}")
                  for k in range(KC)]

            def rope(base):
                # base: [128, BLOCK] bf16; partitions [0:64]=even dims,
                # [64:128]=odd.  rope(x) = x*cos2 + swap(x)*sin2,
                # sin2 = [-sin; +sin], cos2 = [cos; cos].
                sw = ropep.tile([D, BLOCK], BF16, name="sw", tag="sw")
                nc.sync.dma_start(sw[0:64, :], base[64:128, :])
                nc.sync.dma_start(sw[64:128, :], base[0:64, :])
                nc.vector.tensor_mul(sw[:], sw[:], sin_sb[:])
                nc.vector.tensor_mul(base, base, cos_sb[:])
                nc.vector.tensor_add(base, base, sw[:])

            # ---- attention emission helpers (st/pv/sp pools late-bound) ----
            st_pool = None
            stb_pool = None
            pv_pool = None

            def att_scores(h, take):
                """S^T -> exp -> masked expS (bf16), one psum tile per
                j-tile with exp emitted right behind it and `take` filler
                between tiles so the single psum slot never head-of-line
                blocks the PE.  DVE folds the j-tiles into a q-aligned
                bf16 accumulator as they land, so the softmax denominator
                costs the PE nothing."""
                g = h // (HQ // HKV)
                kT = kTa[:, (g % 2) * BLOCK:(g % 2 + 1) * BLOCK]
                qT = qTa[:, h * BLOCK:(h + 1) * BLOCK]
                expS = attsb.tile([128, EW], BF16, name="expS", tag="expS")
                acc = attsb.tile([128, BLOCK], BF16, name="acc", tag="acc",
                                 bufs=2)

                def fold(t):
                    # q-aligned denominator fold (bf16, full DVE lanes)
                    i0 = t * 128
                    if t == 0:
                        nc.vector.tensor_scalar_add(
                            acc[:, 0:128], expS[:, 0:128], 0.0)
                    elif t == 1:
                        nc.vector.tensor_add(
                            acc[:, 128:BLOCK],
                            expS[:, 128:BLOCK],
                            expS[:, OFFS[1]:OFFS[1] + BLOCK - 128])
                    else:
                        nc.vector.tensor_add(
                            acc[:, i0:BLOCK], acc[:, i0:BLOCK],
                            expS[:, OFFS[t]:OFFS[t] + BLOCK - i0])

                for t in range(NI):
                    i0 = t * 128
                    for c in range(2):
                        s0 = max(i0, c * W)
                        s1 = (c + 1) * W
                        if s0 >= s1:
                            continue
                        st = st_pool.tile([128, W], F32, name="st",
                                          tag="st", bufs=3)
                        mm(st[:, 0:s1 - s0],
                           kT[:, i0:i0 + 128], qT[:, s0:s1],
                           start=True, stop=True)
                        nc.scalar.activation(
                            expS[:, OFFS[t] + s0 - i0:OFFS[t] + s1 - i0],
                            st[:, 0:s1 - s0],
                            mybir.ActivationFunctionType.Exp, scale=SCALE)
                        if s0 == i0:
                            # diagonal 128-strip lives in this chunk
                            nc.vector.tensor_mul(
                                expS[:, OFFS[t]:OFFS[t] + 128],
                                expS[:, OFFS[t]:OFFS[t] + 128], maskb[:])
                        if s1 == BLOCK:
                            fold(t)
                        take(3 if t >= 5 else 2)
                return expS, acc

            def att_pv_c(h, expS, c):
                """PV chunk c into a [128, W] psum tile."""
                g = h // (HQ // HKV)
                live = [t for t in range(NI) if t * 128 < (c + 1) * W]
                pv = pv_pool.tile([128, W], F32, name="pv", tag="pv")
                for idx, t in enumerate(live):
                    i0 = t * 128
                    s0 = max(i0, c * W)
                    w = (c + 1) * W - s0
                    e0 = OFFS[t] + (s0 - i0)
                    mm(pv[:, s0 - c * W:s0 - c * W + w],
                       va[:, t * HKV * D + g * D: t * HKV * D + (g + 1) * D],
                       expS[:, e0:e0 + w],
                       start=(idx == 0), stop=(idx == len(live) - 1))
                return pv

            def att_B_open(h, expS, acc):
                """pv0 of head h; denominator all-reduce-broadcast lands
                on GpSimd (two halves so chunk 0 unblocks early); the
                cheap approx reciprocal is deferred to close so it sits
                behind the next head's masks/folds in the DVE FIFO."""
                pv0 = att_pv_c(h, expS, 0)    # 4 mm
                dnb = attsb.tile([128, BLOCK], F32, name="dnb", tag="dnb",
                                 bufs=1)
                for c in range(2):
                    nc.gpsimd.partition_all_reduce(
                        dnb[:, c * W:(c + 1) * W], acc[:, c * W:(c + 1) * W],
                        128, bass_isa.ReduceOp.add)
                return dnb, pv0

            def att_B_close(h, expS, dnb, pv0, take):
                with nc.allow_low_precision("softmax normalize bf16"):
                    nc.vector.reciprocal_approx_fast(
                        dnb[:, 0:W], dnb[:, 0:W])
                    nc.vector.tensor_mul(
                        oTall[:, h * BLOCK: h * BLOCK + W],
                        pv0[:], dnb[:, 0:W])
                    pv1 = att_pv_c(h, expS, 1)    # 8 mm
                    take(4)
                    nc.vector.reciprocal_approx_fast(
                        dnb[:, W:2 * W], dnb[:, W:2 * W])
                    nc.vector.tensor_mul(
                        oTall[:, h * BLOCK + W: h * BLOCK + 2 * W],
                        pv1[:], dnb[:, W:2 * W])

            def emit_att_heads(heads, take):
                """Software-pipelined attention: the normalize stage of
                head h-1 brackets the scores of head h so no engine burst
                sits in front of a PE dependency."""
                prev = None
                for h in heads:
                    if prev is not None:
                        ph, pexp, pacc = prev
                        dnb, pv0 = att_B_open(ph, pexp, pacc)
                        expS, acc = att_scores(h, take)
                        take(2)
                        att_B_close(ph, pexp, dnb, pv0, take)
                        take(7)
                    else:
                        expS, acc = att_scores(h, take)
                        take(8)
                    prev = (h, expS, acc)
                ph, pexp, pacc = prev
                dnb, pv0 = att_B_open(ph, pexp, pacc)
                take(6)
                att_B_close(ph, pexp, dnb, pv0, take)
                take(10)

            def load_wq(g, ck, half):
                # [128, 256] covering heads 4g+2*half, +1
                t = wqp.tile([128, 2 * D], BF16,
                             name=f"wq{g}_{ck}_{half}", tag="wqg")
                c0 = (4 * g + 2 * half) * D
                nc.sync.dma_start(
                    t[:], wq_d[ck * 128:(ck + 1) * 128, c0:c0 + 2 * D])
                return t

            wk_pref = {}

            def prefetch_wk(g):
                wkt = wkp.tile([128, KC * D], BF16, name=f"wk{g}",
                               tag="wkg")
                for s in range(4):
                    nc.sync.dma_start(
                        wkt[:, s * 8 * D:(s + 1) * 8 * D],
                        bass.AP(wk_d.tensor,
                                g * D + s * 8 * 128 * HKV * D,
                                [[HKV * D, 128], [128 * HKV * D, 8],
                                 [1, D]]))
                wk_pref[g] = wkt

            # =================== group 0 (no filler) ===================
            with tc.tile_pool(name="g0ps", bufs=4, space="PSUM") as g0ps:
                wk0 = wkp.tile([128, KC * D], BF16, name="wk0", tag="wkg")
                wv_pre = {}
                qps = {}
                qps[0] = g0ps.tile([128, BLOCK], F32, name="q0ps",
                                   tag="big")
                for j in range(1, 4):
                    qps[j] = g0ps.tile([128, BLOCK], F32, name=f"q{j}ps",
                                       tag="big")
                s1w = {}

                def s1_load(ck):
                    nc.sync.dma_start(xb[ck][:],
                                      xbT_d[ck * 128:(ck + 1) * 128, :])
                    s1w[ck] = (load_wq(0, ck, 0), load_wq(0, ck, 1))
                s1_load(0)
                s1_load(1)
                for ck in range(KC):
                    if ck + 2 < KC:
                        s1_load(ck + 2)
                    wqa, wqb = s1w.pop(ck)
                    if ck == 26:
                        # cos/sin needed at the first rope (end of sweep 1)
                        nc.sync.dma_start(cos_sb[:], cos_d)
                        nc.sync.dma_start(sin_sb[:], sin_d)
                    if ck in (6, 8, 10, 12):
                        # prefetch the k-weight gather early, in quarters so
                        # sweep 1's own loads aren't displaced
                        s = (ck - 6) // 2
                        nc.sync.dma_start(
                            wk0[:, s * 8 * D:(s + 1) * 8 * D],
                            bass.AP(wk_d.tensor,
                                    0 * D + s * 8 * 128 * HKV * D,
                                    [[HKV * D, 128], [128 * HKV * D, 8],
                                     [1, D]]))
                    if ck in (12, 14, 16, 18):
                        pck = (ck - 12) // 2
                        wvt = wvp.tile([128, HKV * D], BF16,
                                       name=f"wv{pck}", tag="wv")
                        nc.sync.dma_start(
                            wvt[:], wv_d[pck * 128:(pck + 1) * 128, :])
                        wv_pre[pck] = wvt
                    for j in range(4):
                        wt = (wqa if j < 2 else wqb)
                        wcol = (j % 2) * D
                        for c in range(2):
                            mm(qps[j][:, c * W:(c + 1) * W],
                               wt[:, wcol:wcol + D],
                               xb[ck][:, c * W:(c + 1) * W],
                               start=(ck == 0), stop=(ck == KC - 1))
                for j in range(4):
                    nc.scalar.copy(qTa[:, j * BLOCK:(j + 1) * BLOCK],
                                          qps[j][:])
                    rope(qTa[:, j * BLOCK:(j + 1) * BLOCK])

                # --- sweep 2: k (qkps) + v tiles 0..5 (g0 pool) ---
                kps = g0ps.tile([128, BLOCK], F32, name="kps", tag="big")
                vps = {}
                for vset in range(3):
                    vps[vset] = g0ps.tile([128, BLOCK], F32,
                                          name=f"vps{vset}", tag="big")
                wv_tiles = {}
                wv2_pre = {}
                for ck in range(KC):
                    if ck in wv_pre:
                        wvt = wv_pre[ck]
                    else:
                        wvt = wvp.tile([128, HKV * D], BF16, name=f"wv{ck}",
                                       tag="wv")
                        nc.sync.dma_start(
                            wvt[:], wv_d[ck * 128:(ck + 1) * 128, :])
                    wv_tiles[ck] = wvt
                    if ck >= 28:
                        rck = ck - 28   # prefetch sweep-3 reloads (ck 0..3)
                        wv2t = wvp.tile([128, HKV * D], BF16,
                                        name=f"wv2_{rck}", tag="wv")
                        nc.sync.dma_start(
                            wv2t[:], wv_d[rck * 128:(rck + 1) * 128, :])
                        wv2_pre[rck] = wv2t
                    for c in range(2):
                        mm(kps[:, c * W:(c + 1) * W],
                           wk0[:, ck * D:(ck + 1) * D],
                           xb[ck][:, c * W:(c + 1) * W],
                           start=(ck == 0), stop=(ck == KC - 1))
                    for t in range(6):
                        mm(vps[t // 2][:, (t % 2) * W:(t % 2) * W + W],
                           xb[ck][:, t * 128:(t + 1) * 128],
                           wvt[:, :HKV * D],
                           start=(ck == 0), stop=(ck == KC - 1))
                nc.scalar.copy(kTa[:, 0:BLOCK], kps[:])
                rope(kTa[:, 0:BLOCK])
                for t in range(6):
                    nc.scalar.copy(
                        va[:, t * HKV * D:(t + 1) * HKV * D],
                        vps[t // 2][:, (t % 2) * W:(t % 2) * W + W])

                # --- sweep 3: v tiles 6,7 (qkps slot).  Chunks 16..31
                # still have their wv tiles resident (wvp bufs=12), so run
                # them first and only reload chunks 0..19. ---
                v67 = g0ps.tile([128, BLOCK], F32, name="v67", tag="big")
                for i3, ck in enumerate(list(range(20, KC)) + list(range(20))):
                    if ck >= 20:
                        wvt = wv_tiles[ck]
                    elif ck in wv2_pre:
                        wvt = wv2_pre[ck]
                    else:
                        wvt = wvp.tile([128, HKV * D], BF16,
                                       name=f"wv2_{ck}", tag="wv")
                        nc.sync.dma_start(
                            wvt[:], wv_d[ck * 128:(ck + 1) * 128, :])
                    # keep 4 reloads in flight
                    pf = ck + 4 if 0 <= ck < 16 else (i3 - 12 + 4 if ck >= 20 and i3 >= 12 else None)
                    for t in (6, 7):
                        mm(v67[:, (t - 6) * W:(t - 6) * W + W],
                           xb[ck][:, t * 128:(t + 1) * 128],
                           wvt[:, :HKV * D],
                           start=(ck == 20), stop=(ck == 19))
                for t in (6, 7):
                    nc.scalar.copy(
                        va[:, t * HKV * D:(t + 1) * HKV * D],
                        v67[:, (t - 6) * W:(t - 6) * W + W])
                prefetch_wk(1)

            # =============== groups 1..3 with interleaved attention =======
            # PSUM budget: scores 3 x [128,512] + qk halves 3 x [128,512]
            # + pv 2 x [128,512] = 8 banks.
            stp = att_ps.enter_context(
                tc.tile_pool(name="stps", bufs=3, space="PSUM"))
            pvsp = att_ps.enter_context(
                tc.tile_pool(name="pvsp", bufs=2, space="PSUM"))
            qkhp = phase1.enter_context(
                tc.tile_pool(name="qkhp", bufs=3, space="PSUM"))
            st_pool = stp
            pv_pool = pvsp

            def group_qkv_ops2(g):
                ops = []
                state = {}

                def proj_half(key, ck, c, wtile, wcol, drain, drain_off):
                    """one [128, W] psum half of a k/q projection; halves
                    rotate in 3 banks so a head's drain never blocks the
                    next head's first matmuls."""
                    if ck == 0:
                        state[key] = qkhp.tile(
                            [128, W], F32, name=f"ps_{key}", tag="qkh")
                    mm(state[key][:],
                       wtile[:, wcol:wcol + D],
                       xb[ck][:, c * W:(c + 1) * W],
                       start=(ck == 0), stop=(ck == KC - 1))
                    if ck == KC - 1:
                        nc.scalar.copy(
                            drain[:, drain_off:drain_off + W],
                            state[key][:])

                for ck in range(KC):
                    for c in range(2):
                        def f(ck=ck, c=c):
                            if ck == 0 and c == 0:
                                state['wk'] = wk_pref[g]
                                # prime the first q-weight loads
                                for pck in range(8):
                                    state[(0, pck)] = load_wq(g, pck, 0)
                            sl = (g % 2) * BLOCK
                            proj_half(f'k{c}', ck, c,
                                      state['wk'][:, ck * D:(ck + 1) * D],
                                      0, kTa, sl + c * W)
                            if ck == KC - 1 and c == 1:
                                rope(kTa[:, sl:sl + BLOCK])
                        ops.append(f)
                for j in range(4):
                    h = 4 * g + j
                    for ck in range(KC):
                        for c in range(2):
                            def f(ck=ck, c=c, j=j, h=h):
                                if c == 0 and j % 2 == 0 and ck + 8 < KC:
                                    state[(j // 2, ck + 8)] = load_wq(
                                        g, ck + 8, j // 2)
                                if c == 0 and j == 1 and ck == 24:
                                    # prefetch next group's k gather
                                    if g + 1 < GROUPS:
                                        prefetch_wk(g + 1)
                                if c == 0 and j == 1 and 23 <= ck < 31:
                                    # prime the j=2/3 pair's first loads
                                    state[(1, ck - 23)] = load_wq(
                                        g, ck - 23, 1)
                                wt = state[(j // 2, ck)]
                                proj_half(f'q{j}_{c}', ck, c,
                                          wt, (j % 2) * D,
                                          qTa, h * BLOCK + c * W)
                                if ck == KC - 1 and c == 1:
                                    rope(qTa[:, h * BLOCK:(h + 1) * BLOCK])
                            ops.append(f)
                return ops

            for g in range(1, GROUPS):
                ops = group_qkv_ops2(g)
                pos = {'i': 0}

                def take(n):
                    e = min(pos['i'] + n, len(ops))
                    for i in range(pos['i'], e):
                        ops[i]()
                    pos['i'] = e
                emit_att_heads([4 * (g - 1) + j for j in range(4)], take)
                take(len(ops))  # remainder

            # xbT / weight pools / qk psum done - free them before the
            # fused tail (attention of group 3 + WO partial pass)
            phase1.close()

            # ---- fused tail: ATT(12..15) with WO pass-0 heads 0..11 as
            # PE filler; partials drain to SBUF bf16 ----
            wow = ctx.enter_context(
                tc.tile_pool(name="wow", bufs=16, side="right"))
            partp = ctx.enter_context(
                tc.tile_pool(name="partp", bufs=1, side="right"))
            parts = {}
            wtsB = {}

            with tc.tile_pool(name="woaps", bufs=2, space="PSUM",
                              side="right") as woaps:
                # two half-loads per head so the q<2 tiles only wait on
                # the first 3MB of weight DMA
                wts0 = {}
                for h in range(12):
                    wt = wow.tile([128, 4 * W], BF16, name=f"woA_{h}",
                                  tag="wot")
                    nc.sync.dma_start(
                        wt[:, 0:2 * W], wo_d[h * D:(h + 1) * D, 0:2 * W])
                    wts0[h] = wt
                for h in range(12):
                    nc.sync.dma_start(
                        wts0[h][:, 2 * W:4 * W],
                        wo_d[h * D:(h + 1) * D, 2 * W:4 * W])
                for h in range(12, HQ):
                    wt = wow.tile([128, 4 * W], BF16, name=f"woB_{h}",
                                  tag="wot")
                    nc.sync.dma_start(
                        wt[:], wo_d[h * D:(h + 1) * D, 0:4 * W])
                    wtsB[h] = wt

                woa_ops = []
                wo_state = {}
                for q in (0, 1, 2, 3):
                    for it in range(NI):
                        for h in range(12):
                            def f(it=it, q=q, h=h):
                                key = (it, q)
                                if h == 0:
                                    wo_state[key] = woaps.tile(
                                        [128, W], F32, name=f"woa{it}_{q}",
                                        tag="woa")
                                o_sl = oTall[:, h * BLOCK + it * 128:
                                             h * BLOCK + it * 128 + 128]
                                mm(wo_state[key][:], o_sl,
                                   wts0[h][:, q * W:(q + 1) * W],
                                   start=(h == 0), stop=(h == 11))
                                if h == 11:
                                    part = partp.tile(
                                        [128, W], BF16,
                                        name=f"part{it}_{q}",
                                        tag=f"part{it}_{q}")
                                    nc.scalar.copy(part[:],
                                                   wo_state[key][:])
                                    parts[key] = part
                            woa_ops.append(f)
                wo_pos = {'i': 0}

                def wo_take(n):
                    e = min(wo_pos['i'] + n, len(woa_ops))
                    for i in range(wo_pos['i'], e):
                        woa_ops[i]()
                    wo_pos['i'] = e

                emit_att_heads([12 + j for j in range(4)], wo_take)
                wo_take(len(woa_ops))  # remainder

            att_ps.close()   # free stps/pvsp

        # ======================= WO finish =======================
        # pass-0 heads 12..15 + saved partials, then full pass 1
        with tc.tile_pool(name="obp", bufs=4) as obp, \
             tc.tile_pool(name="wops", bufs=8, space="PSUM") as wops:
            for it in range(NI):
                ps = [wops.tile([128, W], F32, name=f"wob{it}_{q}",
                                tag="wop") for q in range(4)]
                for h in range(12, HQ):
                    o_sl = oTall[:, h * BLOCK + it * 128:
                                 h * BLOCK + it * 128 + 128]
                    for q in range(4):
                        mm(ps[q][:], o_sl, wtsB[h][:, q * W:(q + 1) * W],
                           start=(h == 12), stop=(h == HQ - 1))
                for q in range(4):
                    ob = obp.tile([128, W], BF16, name="ob", tag="ob")
                    with nc.allow_low_precision("wo partial add"):
                        nc.vector.tensor_add(ob[:], ps[q][:],
                                             parts[(it, q)][:])
                    nc.sync.dma_start(
                        out_d[it * 128:(it + 1) * 128,
                              q * W:(q + 1) * W],
                        ob[:])
            # ---- pass 1: full 16-head accumulation, cols [2048, 4096) ----
            wts1 = {}
            for h in range(HQ):
                wt = wow.tile([128, 4 * W], BF16, name=f"wo1_{h}", tag="wot")
                nc.sync.dma_start(
                    wt[:], wo_d[h * D:(h + 1) * D, 4 * W:8 * W])
                wts1[h] = wt
            for it in range(NI):
                ps = [wops.tile([128, W], F32, name=f"wo1p{it}_{q}",
                                tag="wop") for q in range(4)]
                for h in range(HQ):
                    o_sl = oTall[:, h * BLOCK + it * 128:
                                 h * BLOCK + it * 128 + 128]
                    for q in range(4):
                        mm(ps[q][:], o_sl, wts1[h][:, q * W:(q + 1) * W],
                           start=(h == 0), stop=(h == HQ - 1))
                for q in range(4):
                    ob = obp.tile([128, W], BF16, name="ob", tag="ob")
                    nc.scalar.copy(ob[:], ps[q][:])
                    nc.sync.dma_start(
                        out_d[it * 128:(it + 1) * 128,
                              4 * W + q * W:4 * W + (q + 1) * W],
                        ob[:])

    from concourse.library_overlay import lower_extended_insts
    lower_extended_insts(nc)   # populate .instr for gpsimd extended insts
    _trim_dma_waits(nc)
    import json as _json
    _fixed = _json.dumps(_split_waits_json(
        _json.loads(nc.to_json_bytes()))).encode()
    nc.to_json_bytes = lambda: _fixed
    return nc


def _deinterleave_cols(w, nheads):
    """Per head, reorder the 128 columns to [even head-dims, odd head-dims]."""
    dim = w.shape[0]
    r = w.reshape(dim, nheads, D // 2, 2)
    return np.concatenate([r[..., 0], r[..., 1]], axis=2).reshape(dim, nheads * D)


def shard_inputs(x, wq, wk, wv, wo, freqs_cos, freqs_sin):
    """Build the 8 per-core input maps (core = 2*block + head_group)."""
    import ml_dtypes
    BF = ml_dtypes.bfloat16
    x = np.ascontiguousarray(np.asarray(x, dtype=np.float32))
    wq_p = _deinterleave_cols(np.asarray(wq, dtype=np.float32), 32)
    wk_p = _deinterleave_cols(np.asarray(wk, dtype=np.float32), 8)
    wv = np.asarray(wv, dtype=np.float32)
    wo = np.asarray(wo, dtype=np.float32)
    cos = np.asarray(freqs_cos, dtype=np.float32)
    sin = np.asarray(freqs_sin, dtype=np.float32)

    wq_h = wq_p.reshape(DIM, 32, D)
    wk_h = wk_p.reshape(DIM, 8, D)
    wv_h = wv.reshape(DIM, 8, D)
    wo_h = wo.reshape(32, D, DIM)

    in_maps = []
    for core in range(N_CORES):
        b, g = divmod(core, 2)
        rows = slice(b * BLOCK, (b + 1) * BLOCK)
        cosT = cos[rows].T                       # [64, block]
        sinT = sin[rows].T
        cos2 = np.concatenate([cosT, cosT], axis=0)     # [128, block]
        sin2 = np.concatenate([-sinT, sinT], axis=0)
        in_maps.append({
            "xbT": np.ascontiguousarray(x[rows, :].T).astype(BF),
            "wq": np.ascontiguousarray(
                wq_h[:, g * HQ:(g + 1) * HQ].reshape(DIM, HQ * D)).astype(BF),
            "wk": np.ascontiguousarray(
                wk_h[:, g * HKV:(g + 1) * HKV].reshape(DIM, HKV * D)).astype(BF),
            "wv": np.ascontiguousarray(
                wv_h[:, g * HKV:(g + 1) * HKV].reshape(DIM, HKV * D)).astype(BF),
            "wo": np.ascontiguousarray(
                wo_h[g * HQ:(g + 1) * HQ].reshape(HQ * D, DIM)).astype(BF),
            "cos2": np.ascontiguousarray(cos2).astype(BF),
            "sin2": np.ascontiguousarray(sin2).astype(BF),
        })
    return in_maps


def unshard_output(core_outs):
    full = np.empty((NB_TOTAL, DIM), dtype=np.float32)
    for b in range(NB_TOTAL // BLOCK):
        full[b * BLOCK:(b + 1) * BLOCK] = \
            np.asarray(core_outs[2 * b], dtype=np.float32) + \
            np.asarray(core_outs[2 * b + 1], dtype=np.float32)
    return full


NB_TOTAL = 4096  # total sequence length

_NC_CACHE = {}


def _get_nc():
    key = (DIM, BLOCK, HQ, HKV)
    if key not in _NC_CACHE:
        _NC_CACHE[key] = build_kernel()
    return _NC_CACHE[key]


def kernel(x, wq, wk, wv, wo, freqs_cos, freqs_sin, block_size, **run_kwargs):
    assert int(block_size) == BLOCK, f"unexpected block_size {block_size}"
    in_maps = shard_inputs(x, wq, wk, wv, wo, freqs_cos, freqs_sin)
    nc = _get_nc()
    res = bass_utils.run_bass_kernel_spmd(
        nc, in_maps, core_ids=list(range(N_CORES)), **run_kwargs)
    outs = [r["out"] for r in res.results]
    out = unshard_output(outs)
    kernel.last_results = res
    return out



# revision 26
# speedup vs baseline: 1.0155x; 1.0155x over previous
"""Block-diagonal causal GQA attention with RoPE, sharded over 8 TRN2 cores.

Problem (hardcoded from the spec):
  x [4096, 4096], wq [4096, 4096] (32 q heads x 128), wk/wv [4096, 1024]
  (8 kv heads), wo [4096, 4096], freqs_cos/sin [4096, 64], block_size 1024.
  4 independent causal blocks of 1024 tokens.

Sharding: 8 cores = 4 sequence blocks x 2 head-groups.  Core (b, g)
computes block b for q-heads [16g, 16g+16) (kv heads [4g, 4g+4)) and the
partial output projection through the matching rows of wo.  The host sums
the two head-group partials per block and concatenates the blocks.

v2 design (vs the fp32r baseline):
  - all matmul operands bf16 (hosts converts); psum stays f32.  bf16 is
    1 cyc/row at any width (fp32r degrades 4x below 256) and halves DMA.
  - single fused PE stream: 4 kv-groups, each group = [k sweep, 4 q
    sweeps] over the full resident xbT; the attention of group g-1 is
    interleaved into group g's sweeps as filler so the PE never idles
    waiting on ACT exp / DVE reciprocal chains.
  - causal mask applied POST-exp as a bf16 mask multiply (DVE) so the
    S->exp critical path has no DVE hop before ACT.
  - denominator via ones-matmul on the trapezoid expS layout; broadcast
    of the sum via K=1 matmul; reciprocal on [128,512] (full DVE lanes).
  - WO restructured: stationary oT slice held for 4 matmuls (nch quads),
    wo streamed bf16 per half-pass.
"""

import numpy as np
from contextlib import ExitStack

import concourse.bass as bass
import concourse.bass_isa as bass_isa
import concourse.tile as tile
import concourse.mybir as mybir
from concourse import bass_utils, library_config

F32 = mybir.dt.float32
BF16 = mybir.dt.bfloat16

DIM = 4096
BLOCK = 1024
D = 128            # head dim
HQ = 16            # q heads per core
HKV = 4            # kv heads per core
GROUPS = 4         # kv groups per core (rep = HQ // HKV)
N_CORES = 8
NEG = -1.0e9
W = 512            # psum bank width (f32)
NI = BLOCK // 128  # j-tiles per block (8)
KC = DIM // 128    # contraction chunks (32)
SCALE = float(1.0 / np.sqrt(D))

# bf16 1.0 pair packed as f32 bits, for memset on bf16 tiles
BF16_ONES_F32 = float(np.array([0x3F803F80], dtype=np.uint32).view(np.float32)[0])


def _trim_dma_waits(nc):
    """Drop DMA semaphore waits that are transitively guaranteed.

    The DGE descriptor path supports only 2 sync-wait commands per DMA,
    but Tile's wait emission is not transitively minimal.  We compute,
    for every instruction, a conservative "floor": the semaphore values
    guaranteed to have been reached by the time it completes (its own
    waits, the floors of the instructions those waits observe, the
    floors of its sync dependencies, plus in-order completion along each
    semaphore's single FIFO ring).  A wait on a DMA is dead if the
    floors implied by its remaining waits already cover it.
    """
    import bass_rust

    insts = []
    for blk in nc.m.functions[0].blocks:
        insts.extend(blk.instructions)

    floors: dict[str, dict[int, int]] = {}     # inst name -> {sem id: value}
    chain: dict[int, list[tuple[int, str]]] = {}  # sem id -> [(post_val, name)]
    cum: dict[int, int] = {}

    def sem_floor(sem_id, v):
        lst = chain.get(sem_id)
        if not lst:
            return None
        import bisect
        idx = bisect.bisect_left(lst, (v, ""))
        if idx == len(lst):
            return None
        return floors.get(lst[idx][1])

    def merge(dst, src):
        if not src:
            return
        for k, v in src.items():
            if dst.get(k, -1) < v:
                dst[k] = v

    for ins in insts:
        si = ins.sync_info
        fl: dict[int, int] = {}
        if si is not None:
            for w in si.on_wait:
                if w.wait_mode != "sem-ge-imm" or w.wait_value is None:
                    continue
                if fl.get(w.id, -1) < w.wait_value:
                    fl[w.id] = w.wait_value
                merge(fl, sem_floor(w.id, w.wait_value))
        try:
            for dn in ins.sync_dependency_names():
                merge(fl, floors.get(dn))
        except TypeError:
            pass
        if si is not None:
            for u in si.on_update:
                if u.update_mode not in ("sem-add-imm", "sem-inc") \
                        or u.update_value is None:
                    continue
                post = cum.get(u.id, 0) + u.update_value
                cum[u.id] = post
                lst = chain.setdefault(u.id, [])
                if lst:
                    merge(fl, floors.get(lst[-1][1]))
                if fl.get(u.id, -1) < post:
                    fl[u.id] = post
                lst.append((post, ins.name))
        floors[ins.name] = fl

    for ins in insts:
        if not isinstance(ins, mybir.InstDMACopy):
            continue
        si = ins.sync_info
        if si is None:
            continue
        waits = list(si.on_wait)
        changed = True
        while len(waits) > 1 and changed:
            changed = False
            for i, w in enumerate(waits):
                if w.wait_mode != "sem-ge-imm" or w.wait_value is None:
                    continue
                implied: dict[int, int] = {}
                for j, w2 in enumerate(waits):
                    if j == i or w2.wait_mode != "sem-ge-imm":
                        continue
                    merge(implied, sem_floor(w2.id, w2.wait_value))
                if implied.get(w.id, -1) >= w.wait_value:
                    waits.pop(i)
                    changed = True
                    break
        if len(waits) != len(si.on_wait):
            ins.sync_info = bass_rust.SyncInfo(
                on_wait=waits, on_update=list(si.on_update))


def _split_waits_json(bir):
    """Split multi-wait instructions at the BIR level.

    walrus' setupSyncWait budget: one wait of any value, or two waits
    whose values both fit a one-byte command.  Excess waits move onto
    standalone EventSemaphore instructions inserted directly before the
    instruction on the same engine.
    """
    nid = 0
    for fn in bir["functions"]:
        for blk in fn["blocks"]:
            out = []
            for ins in blk["instructions"]:
                si = ins.get("sync_info")
                waits = (si or {}).get("on_wait") or []
                if len(waits) > 1:
                    waits = sorted(
                        waits, key=lambda w: -(w.get("wait_value") or 0))
                    for w in waits[1:]:
                        nid += 1
                        out.append({
                            "debug": ins.get("debug"),
                            "engine": ins["engine"],
                            "ins": [],
                            "outs": [],
                            "name": f"{ins['name']}-w{nid}",
                            "opcode": "EventSemaphore",
                            "sync_info": {"on_update": [], "on_wait": [w]},
                        })
                    si["on_wait"] = waits[:1]
                out.append(ins)
            blk["instructions"] = out
    return bir


# expS free-dim trapezoid layout: j-tile t occupies
# [OFFS[t], OFFS[t] + BLOCK - 128 t)
OFFS = []
_o = 0
for _t in range(NI):
    OFFS.append(_o)
    _o += BLOCK - _t * 128
EW = _o  # 4608


def build_kernel():
    nc = bass.Bass("TRN2", target_bir_lowering=False, debug=False)

    xbT_d = nc.dram_tensor("xbT", [DIM, BLOCK], BF16, kind="ExternalInput").ap()
    wq_d = nc.dram_tensor("wq", [DIM, HQ * D], BF16, kind="ExternalInput").ap()
    wk_d = nc.dram_tensor("wk", [DIM, HKV * D], BF16, kind="ExternalInput").ap()
    wv_d = nc.dram_tensor("wv", [DIM, HKV * D], BF16, kind="ExternalInput").ap()
    wo_d = nc.dram_tensor("wo", [HQ * D, DIM], BF16, kind="ExternalInput").ap()
    cos_d = nc.dram_tensor("cos2", [D, BLOCK], BF16, kind="ExternalInput").ap()
    sin_d = nc.dram_tensor("sin2", [D, BLOCK], BF16, kind="ExternalInput").ap()
    out_d = nc.dram_tensor("out", [BLOCK, DIM], BF16, kind="ExternalOutput").ap()

    def mm(out_ap, lhsT, rhs, **kw):
        nc.tensor.matmul(out_ap, lhsT, rhs, **kw)

    with tile.TileContext(nc) as tc, ExitStack() as ctx:
        const = ctx.enter_context(tc.tile_pool(name="const", bufs=1))
        # softmax denominator runs on GpSimd custom ops (attn library)
        nc.gpsimd.load_library(library_config.attn)
        # bf16 causal mask for the diagonal 128x128 strip of each S^T
        # j-tile: keep (1.0) where i_local >= j_local else 0.0
        tri_f = const.tile([128, 128], F32)
        nc.gpsimd.memset(tri_f[:], 1.0)
        nc.gpsimd.affine_select(
            out=tri_f[:], in_=tri_f[:],
            compare_op=mybir.AluOpType.is_ge,
            fill=0.0, base=0, pattern=[[1, 128]], channel_multiplier=-1,
        )
        maskb = const.tile([128, 128], BF16)
        nc.scalar.copy(maskb[:], tri_f[:])

        # O^T persists through attention into the WO phase
        oT_pool = ctx.enter_context(
            tc.tile_pool(name="oTp", bufs=1, side="right"))
        oTall = oT_pool.tile([128, HQ * BLOCK], BF16, name="oTall")

        att_ps = ExitStack()      # stps/pvsp: closed manually after tail
        phase1 = ExitStack()      # x/weight pools + qk psum: closed after groups

        with tc.tile_pool(name="accs", bufs=1) as accs, \
             tc.tile_pool(name="attsb", bufs=2) as attsb:

            qTa = accs.tile([128, HQ * BLOCK], BF16, name="qTa")
            kTa = accs.tile([128, 2 * BLOCK], BF16, name="kTa")
            va = accs.tile([128, NI * HKV * D], BF16, name="va")
            cos_sb = accs.tile([D, BLOCK], BF16, name="cos_sb")
            sin_sb = accs.tile([D, BLOCK], BF16, name="sin_sb")

            xbp = phase1.enter_context(tc.tile_pool(name="xbp", bufs=1))
            wqp = phase1.enter_context(tc.tile_pool(name="wqp", bufs=32))
            wkp = phase1.enter_context(tc.tile_pool(name="wkp", bufs=1))
            wvp = phase1.enter_context(tc.tile_pool(name="wvp", bufs=11))
            ropep = phase1.enter_context(tc.tile_pool(name="ropep", bufs=1))

            xb = [xbp.tile([128, BLOCK], BF16, name=f"xb{k}", tag=f"xb{k}")
                  for k in range(KC)]

            def rope(base):
                # base: [128, BLOCK] bf16; partitions [0:64]=even dims,
                # [64:128]=odd.  rope(x) = x*cos2 + swap(x)*sin2,
                # sin2 = [-sin; +sin], cos2 = [cos; cos].
                sw = ropep.tile([D, BLOCK], BF16, name="sw", tag="sw")
                nc.sync.dma_start(sw[0:64, :], base[64:128, :])
                nc.sync.dma_start(sw[64:128, :], base[0:64, :])
                nc.vector.tensor_mul(sw[:], sw[:], sin_sb[:])
                nc.vector.tensor_mul(base, base, cos_sb[:])
                nc.vector.tensor_add(base, base, sw[:])

            # ---- attention emission helpers (st/pv/sp pools late-bound) ----
            st_pool = None
            stb_pool = None
            pv_pool = None

            def att_scores(h, take):
                """S^T -> exp -> masked expS (bf16), one psum tile per
                j-tile with exp emitted right behind it and `take` filler
                between tiles so the single psum slot never head-of-line
                blocks the PE.  DVE folds the j-tiles into a q-aligned
                bf16 accumulator as they land, so the softmax denominator
                costs the PE nothing."""
                g = h // (HQ // HKV)
                kT = kTa[:, (g % 2) * BLOCK:(g % 2 + 1) * BLOCK]
                qT = qTa[:, h * BLOCK:(h + 1) * BLOCK]
                expS = attsb.tile([128, EW], BF16, name="expS", tag="expS")
                acc = attsb.tile([128, BLOCK], BF16, name="acc", tag="acc",
                                 bufs=2)

                def fold(t):
                    # q-aligned denominator fold (bf16, full DVE lanes)
                    i0 = t * 128
                    if t == 0:
                        nc.vector.tensor_scalar_add(
                            acc[:, 0:128], expS[:, 0:128], 0.0)
                    elif t == 1:
                        nc.vector.tensor_add(
                            acc[:, 128:BLOCK],
                            expS[:, 128:BLOCK],
                            expS[:, OFFS[1]:OFFS[1] + BLOCK - 128])
                    else:
                        nc.vector.tensor_add(
                            acc[:, i0:BLOCK], acc[:, i0:BLOCK],
                            expS[:, OFFS[t]:OFFS[t] + BLOCK - i0])

                for t in range(NI):
                    i0 = t * 128
                    for c in range(2):
                        s0 = max(i0, c * W)
                        s1 = (c + 1) * W
                        if s0 >= s1:
                            continue
                        st = st_pool.tile([128, W], F32, name="st",
                                          tag="st", bufs=3)
                        mm(st[:, 0:s1 - s0],
                           kT[:, i0:i0 + 128], qT[:, s0:s1],
                           start=True, stop=True)
                        nc.scalar.activation(
                            expS[:, OFFS[t] + s0 - i0:OFFS[t] + s1 - i0],
                            st[:, 0:s1 - s0],
                            mybir.ActivationFunctionType.Exp, scale=SCALE)
                        if s0 == i0:
                            # diagonal 128-strip lives in this chunk
                            nc.vector.tensor_mul(
                                expS[:, OFFS[t]:OFFS[t] + 128],
                                expS[:, OFFS[t]:OFFS[t] + 128], maskb[:])
                        if s1 == BLOCK:
                            fold(t)
                        take(2)
                return expS, acc

            def att_pv_c(h, expS, c):
                """PV chunk c into a [128, W] psum tile."""
                g = h // (HQ // HKV)
                live = [t for t in range(NI) if t * 128 < (c + 1) * W]
                pv = pv_pool.tile([128, W], F32, name="pv", tag="pv")
                for idx, t in enumerate(live):
                    i0 = t * 128
                    s0 = max(i0, c * W)
                    w = (c + 1) * W - s0
                    e0 = OFFS[t] + (s0 - i0)
                    mm(pv[:, s0 - c * W:s0 - c * W + w],
                       va[:, t * HKV * D + g * D: t * HKV * D + (g + 1) * D],
                       expS[:, e0:e0 + w],
                       start=(idx == 0), stop=(idx == len(live) - 1))
                return pv

            def att_B_open(h, expS, acc):
                """pv0 of head h; denominator all-reduce-broadcast lands
                on GpSimd (two halves so chunk 0 unblocks early); the
                cheap approx reciprocal is deferred to close so it sits
                behind the next head's masks/folds in the DVE FIFO."""
                pv0 = att_pv_c(h, expS, 0)    # 4 mm
                dnb = attsb.tile([128, BLOCK], F32, name="dnb", tag="dnb",
                                 bufs=1)
                for c in range(2):
                    nc.gpsimd.partition_all_reduce(
                        dnb[:, c * W:(c + 1) * W], acc[:, c * W:(c + 1) * W],
                        128, bass_isa.ReduceOp.add)
                return dnb, pv0

            def att_B_close(h, expS, dnb, pv0, take):
                with nc.allow_low_precision("softmax normalize bf16"):
                    nc.vector.reciprocal_approx_fast(
                        dnb[:, 0:W], dnb[:, 0:W])
                    nc.vector.tensor_mul(
                        oTall[:, h * BLOCK: h * BLOCK + W],
                        pv0[:], dnb[:, 0:W])
                    pv1 = att_pv_c(h, expS, 1)    # 8 mm
                    take(4)
                    nc.vector.reciprocal_approx_fast(
                        dnb[:, W:2 * W], dnb[:, W:2 * W])
                    nc.vector.tensor_mul(
                        oTall[:, h * BLOCK + W: h * BLOCK + 2 * W],
                        pv1[:], dnb[:, W:2 * W])

            def emit_att_heads(heads, take):
                """Software-pipelined attention: the normalize stage of
                head h-1 brackets the scores of head h so no engine burst
                sits in front of a PE dependency."""
                prev = None
                for h in heads:
                    if prev is not None:
                        ph, pexp, pacc = prev
                        dnb, pv0 = att_B_open(ph, pexp, pacc)
                        expS, acc = att_scores(h, take)
                        take(2)
                        att_B_close(ph, pexp, dnb, pv0, take)
                        take(10)
                    else:
                        expS, acc = att_scores(h, take)
                        take(8)
                    prev = (h, expS, acc)
                ph, pexp, pacc = prev
                dnb, pv0 = att_B_open(ph, pexp, pacc)
                take(6)
                att_B_close(ph, pexp, dnb, pv0, take)
                take(10)

            def load_wq(g, ck, half):
                # [128, 256] covering heads 4g+2*half, +1
                t = wqp.tile([128, 2 * D], BF16,
                             name=f"wq{g}_{ck}_{half}", tag="wqg")
                c0 = (4 * g + 2 * half) * D
                nc.sync.dma_start(
                    t[:], wq_d[ck * 128:(ck + 1) * 128, c0:c0 + 2 * D])
                return t

            wk_pref = {}

            def prefetch_wk(g):
                wkt = wkp.tile([128, KC * D], BF16, name=f"wk{g}",
                               tag="wkg")
                for s in range(4):
                    nc.sync.dma_start(
                        wkt[:, s * 8 * D:(s + 1) * 8 * D],
                        bass.AP(wk_d.tensor,
                                g * D + s * 8 * 128 * HKV * D,
                                [[HKV * D, 128], [128 * HKV * D, 8],
                                 [1, D]]))
                wk_pref[g] = wkt

            # =================== group 0 (no filler) ===================
            with tc.tile_pool(name="g0ps", bufs=4, space="PSUM") as g0ps:
                wk0 = wkp.tile([128, KC * D], BF16, name="wk0", tag="wkg")
                wv_pre = {}
                qps = {}
                qps[0] = g0ps.tile([128, BLOCK], F32, name="q0ps",
                                   tag="big")
                for j in range(1, 4):
                    qps[j] = g0ps.tile([128, BLOCK], F32, name=f"q{j}ps",
                                       tag="big")
                s1w = {}

                def s1_load(ck):
                    nc.sync.dma_start(xb[ck][:],
                                      xbT_d[ck * 128:(ck + 1) * 128, :])
                    s1w[ck] = (load_wq(0, ck, 0), load_wq(0, ck, 1))
                s1_load(0)
                s1_load(1)
                for ck in range(KC):
                    if ck + 2 < KC:
                        s1_load(ck + 2)
                    wqa, wqb = s1w.pop(ck)
                    if ck == 26:
                        # cos/sin needed at the first rope (end of sweep 1)
                        nc.sync.dma_start(cos_sb[:], cos_d)
                        nc.sync.dma_start(sin_sb[:], sin_d)
                    if ck in (6, 8, 10, 12):
                        # prefetch the k-weight gather early, in quarters so
                        # sweep 1's own loads aren't displaced
                        s = (ck - 6) // 2
                        nc.sync.dma_start(
                            wk0[:, s * 8 * D:(s + 1) * 8 * D],
                            bass.AP(wk_d.tensor,
                                    0 * D + s * 8 * 128 * HKV * D,
                                    [[HKV * D, 128], [128 * HKV * D, 8],
                                     [1, D]]))
                    if ck in (12, 14, 16, 18):
                        pck = (ck - 12) // 2
                        wvt = wvp.tile([128, HKV * D], BF16,
                                       name=f"wv{pck}", tag="wv")
                        nc.sync.dma_start(
                            wvt[:], wv_d[pck * 128:(pck + 1) * 128, :])
                        wv_pre[pck] = wvt
                    for j in range(4):
                        wt = (wqa if j < 2 else wqb)
                        wcol = (j % 2) * D
                        for c in range(2):
                            mm(qps[j][:, c * W:(c + 1) * W],
                               wt[:, wcol:wcol + D],
                               xb[ck][:, c * W:(c + 1) * W],
                               start=(ck == 0), stop=(ck == KC - 1))
                for j in range(4):
                    nc.scalar.copy(qTa[:, j * BLOCK:(j + 1) * BLOCK],
                                          qps[j][:])
                    rope(qTa[:, j * BLOCK:(j + 1) * BLOCK])

                # --- sweep 2: k (qkps) + v tiles 0..5 (g0 pool) ---
                kps = g0ps.tile([128, BLOCK], F32, name="kps", tag="big")
                vps = {}
                for vset in range(3):
                    vps[vset] = g0ps.tile([128, BLOCK], F32,
                                          name=f"vps{vset}", tag="big")
                wv_tiles = {}
                wv2_pre = {}
                for ck in range(KC):
                    if ck in wv_pre:
                        wvt = wv_pre[ck]
                    else:
                        wvt = wvp.tile([128, HKV * D], BF16, name=f"wv{ck}",
                                       tag="wv")
                        nc.sync.dma_start(
                            wvt[:], wv_d[ck * 128:(ck + 1) * 128, :])
                    wv_tiles[ck] = wvt
                    if ck >= 28:
                        rck = ck - 28   # prefetch sweep-3 reloads (ck 0..3)
                        wv2t = wvp.tile([128, HKV * D], BF16,
                                        name=f"wv2_{rck}", tag="wv")
                        nc.sync.dma_start(
                            wv2t[:], wv_d[rck * 128:(rck + 1) * 128, :])
                        wv2_pre[rck] = wv2t
                    for c in range(2):
                        mm(kps[:, c * W:(c + 1) * W],
                           wk0[:, ck * D:(ck + 1) * D],
                           xb[ck][:, c * W:(c + 1) * W],
                           start=(ck == 0), stop=(ck == KC - 1))
                    for t in range(6):
                        mm(vps[t // 2][:, (t % 2) * W:(t % 2) * W + W],
                           xb[ck][:, t * 128:(t + 1) * 128],
                           wvt[:, :HKV * D],
                           start=(ck == 0), stop=(ck == KC - 1))
                nc.scalar.copy(kTa[:, 0:BLOCK], kps[:])
                rope(kTa[:, 0:BLOCK])
                for t in range(6):
                    nc.scalar.copy(
                        va[:, t * HKV * D:(t + 1) * HKV * D],
                        vps[t // 2][:, (t % 2) * W:(t % 2) * W + W])

                # --- sweep 3: v tiles 6,7 (qkps slot).  Chunks 16..31
                # still have their wv tiles resident (wvp bufs=12), so run
                # them first and only reload chunks 0..19. ---
                v67 = g0ps.tile([128, BLOCK], F32, name="v67", tag="big")
                for i3, ck in enumerate(list(range(20, KC)) + list(range(20))):
                    if ck >= 20:
                        wvt = wv_tiles[ck]
                    elif ck in wv2_pre:
                        wvt = wv2_pre[ck]
                    else:
                        wvt = wvp.tile([128, HKV * D], BF16,
                                       name=f"wv2_{ck}", tag="wv")
                        nc.sync.dma_start(
                            wvt[:], wv_d[ck * 128:(ck + 1) * 128, :])
                    # keep 4 reloads in flight
                    pf = ck + 4 if 0 <= ck < 16 else (i3 - 12 + 4 if ck >= 20 and i3 >= 12 else None)
                    for t in (6, 7):
                        mm(v67[:, (t - 6) * W:(t - 6) * W + W],
                           xb[ck][:, t * 128:(t + 1) * 128],
                           wvt[:, :HKV * D],
                           start=(ck == 20), stop=(ck == 19))
                for t in (6, 7):
                    nc.scalar.copy(
                        va[:, t * HKV * D:(t + 1) * HKV * D],
                        v67[:, (t - 6) * W:(t - 6) * W + W])
                prefetch_wk(1)

            # =============== groups 1..3 with interleaved attention =======
            # PSUM budget: scores 3 x [128,512] + qk halves 3 x [128,512]
            # + pv 2 x [128,512] = 8 banks.
            stp = att_ps.enter_context(
                tc.tile_pool(name="stps", bufs=3, space="PSUM"))
            pvsp = att_ps.enter_context(
                tc.tile_pool(name="pvsp", bufs=2, space="PSUM"))
            qkhp = phase1.enter_context(
                tc.tile_pool(name="qkhp", bufs=3, space="PSUM"))
            st_pool = stp
            pv_pool = pvsp

            def group_qkv_ops2(g):
                ops = []
                state = {}

                def proj_half(key, ck, c, wtile, wcol, drain, drain_off):
                    """one [128, W] psum half of a k/q projection; halves
                    rotate in 3 banks so a head's drain never blocks the
                    next head's first matmuls."""
                    if ck == 0:
                        state[key] = qkhp.tile(
                            [128, W], F32, name=f"ps_{key}", tag="qkh")
                    mm(state[key][:],
                       wtile[:, wcol:wcol + D],
                       xb[ck][:, c * W:(c + 1) * W],
                       start=(ck == 0), stop=(ck == KC - 1))
                    if ck == KC - 1:
                        nc.scalar.copy(
                            drain[:, drain_off:drain_off + W],
                            state[key][:])

                for ck in range(KC):
                    for c in range(2):
                        def f(ck=ck, c=c):
                            if ck == 0 and c == 0:
                                state['wk'] = wk_pref[g]
                                # prime the first q-weight loads
                                for pck in range(4):
                                    state[(0, pck)] = load_wq(g, pck, 0)
                            sl = (g % 2) * BLOCK
                            proj_half(f'k{c}', ck, c,
                                      state['wk'][:, ck * D:(ck + 1) * D],
                                      0, kTa, sl + c * W)
                            if ck == KC - 1 and c == 1:
                                rope(kTa[:, sl:sl + BLOCK])
                        ops.append(f)
                for j in range(4):
                    h = 4 * g + j
                    for ck in range(KC):
                        for c in range(2):
                            def f(ck=ck, c=c, j=j, h=h):
                                if c == 0 and j % 2 == 0 and ck + 4 < KC:
                                    state[(j // 2, ck + 4)] = load_wq(
                                        g, ck + 4, j // 2)
                                if c == 0 and j == 1 and ck == 24:
                                    # prefetch next group's k gather
                                    if g + 1 < GROUPS:
                                        prefetch_wk(g + 1)
                                if c == 0 and j == 1 and 27 <= ck < 31:
                                    # prime the j=2/3 pair's first loads
                                    state[(1, ck - 27)] = load_wq(
                                        g, ck - 27, 1)
                                wt = state[(j // 2, ck)]
                                proj_half(f'q{j}_{c}', ck, c,
                                          wt, (j % 2) * D,
                                          qTa, h * BLOCK + c * W)
                                if ck == KC - 1 and c == 1:
                                    rope(qTa[:, h * BLOCK:(h + 1) * BLOCK])
                            ops.append(f)
                return ops

            for g in range(1, GROUPS):
                ops = group_qkv_ops2(g)
                pos = {'i': 0}

                def take(n):
                    e = min(pos['i'] + n, len(ops))
                    for i in range(pos['i'], e):
                        ops[i]()
                    pos['i'] = e
                emit_att_heads([4 * (g - 1) + j for j in range(4)], take)
                take(len(ops))  # remainder

            # xbT / weight pools / qk psum done - free them before the
            # fused tail (attention of group 3 + WO partial pass)
            phase1.close()

            # ---- fused tail: ATT(12..15) with WO pass-0 heads 0..11 as
            # PE filler; partials drain to SBUF bf16 ----
            wow = ctx.enter_context(
                tc.tile_pool(name="wow", bufs=16, side="right"))
            partp = ctx.enter_context(
                tc.tile_pool(name="partp", bufs=1, side="right"))
            parts = {}
            wtsB = {}

            with tc.tile_pool(name="woaps", bufs=2, space="PSUM",
                              side="right") as woaps:
                # two half-loads per head so the q<2 tiles only wait on
                # the first 3MB of weight DMA
                wts0 = {}
                for h in range(12):
                    wt = wow.tile([128, 4 * W], BF16, name=f"woA_{h}",
                                  tag="wot")
                    nc.sync.dma_start(
                        wt[:, 0:2 * W], wo_d[h * D:(h + 1) * D, 0:2 * W])
                    wts0[h] = wt
                for h in range(12):
                    nc.sync.dma_start(
                        wts0[h][:, 2 * W:4 * W],
                        wo_d[h * D:(h + 1) * D, 2 * W:4 * W])
                for h in range(12, HQ):
                    wt = wow.tile([128, 4 * W], BF16, name=f"woB_{h}",
                                  tag="wot")
                    nc.sync.dma_start(
                        wt[:], wo_d[h * D:(h + 1) * D, 0:4 * W])
                    wtsB[h] = wt

                woa_ops = []
                wo_state = {}
                for q in (0, 1, 2, 3):
                    for it in range(NI):
                        for h in range(12):
                            def f(it=it, q=q, h=h):
                                key = (it, q)
                                if h == 0:
                                    wo_state[key] = woaps.tile(
                                        [128, W], F32, name=f"woa{it}_{q}",
                                        tag="woa")
                                o_sl = oTall[:, h * BLOCK + it * 128:
                                             h * BLOCK + it * 128 + 128]
                                mm(wo_state[key][:], o_sl,
                                   wts0[h][:, q * W:(q + 1) * W],
                                   start=(h == 0), stop=(h == 11))
                                if h == 11:
                                    part = partp.tile(
                                        [128, W], BF16,
                                        name=f"part{it}_{q}",
                                        tag=f"part{it}_{q}")
                                    nc.scalar.copy(part[:],
                                                   wo_state[key][:])
                                    parts[key] = part
                            woa_ops.append(f)
                wo_pos = {'i': 0}

                def wo_take(n):
                    e = min(wo_pos['i'] + n, len(woa_ops))
                    for i in range(wo_pos['i'], e):
                        woa_ops[i]()
                    wo_pos['i'] = e

                emit_att_heads([12 + j for j in range(4)], wo_take)
                wo_take(len(woa_ops))  # remainder

            att_ps.close()   # free stps/pvsp

        # ======================= WO finish =======================
        # pass-0 heads 12..15 + saved partials, then full pass 1
        with tc.tile_pool(name="obp", bufs=4) as obp, \
             tc.tile_pool(name="wops", bufs=8, space="PSUM") as wops:
            for it in range(NI):
                ps = [wops.tile([128, W], F32, name=f"wob{it}_{q}",
                                tag="wop") for q in range(4)]
                for h in range(12, HQ):
                    o_sl = oTall[:, h * BLOCK + it * 128:
                                 h * BLOCK + it * 128 + 128]
                    for q in range(4):
                        mm(ps[q][:], o_sl, wtsB[h][:, q * W:(q + 1) * W],
                           start=(h == 12), stop=(h == HQ - 1))
                for q in range(4):
                    ob = obp.tile([128, W], BF16, name="ob", tag="ob")
                    with nc.allow_low_precision("wo partial add"):
                        nc.vector.tensor_add(ob[:], ps[q][:],
                                             parts[(it, q)][:])
                    nc.sync.dma_start(
                        out_d[it * 128:(it + 1) * 128,
                              q * W:(q + 1) * W],
                        ob[:])
            # ---- pass 1: full 16-head accumulation, cols [2048, 4096) ----
            wts1 = {}
            for h in range(HQ):
                wt = wow.tile([128, 4 * W], BF16, name=f"wo1_{h}", tag="wot")
                nc.sync.dma_start(
                    wt[:], wo_d[h * D:(h + 1) * D, 4 * W:8 * W])
                wts1[h] = wt
            for it in range(NI):
                ps = [wops.tile([128, W], F32, name=f"wo1p{it}_{q}",
                                tag="wop") for q in range(4)]
                for h in range(HQ):
                    o_sl = oTall[:, h * BLOCK + it * 128:
                                 h * BLOCK + it * 128 + 128]
                    for q in range(4):
                        mm(ps[q][:], o_sl, wts1[h][:, q * W:(q + 1) * W],
                           start=(h == 0), stop=(h == HQ - 1))
                for q in range(4):
                    ob = obp.tile([128, W], BF16, name="ob", tag="ob")
                    nc.scalar.copy(ob[:], ps[q][:])
                    nc.sync.dma_start(
                        out_d[it * 128:(it + 1) * 128,
                              4 * W + q * W:4 * W + (q + 1) * W],
                        ob[:])

    from concourse.library_overlay import lower_extended_insts
    lower_extended_insts(nc)   # populate .instr for gpsimd extended insts
    _trim_dma_waits(nc)
    import json as _json
    _fixed = _json.dumps(_split_waits_json(
        _json.loads(nc.to_json_bytes()))).encode()
    nc.to_json_bytes = lambda: _fixed
    return nc


def _deinterleave_cols(w, nheads):
    """Per head, reorder the 128 columns to [even head-dims, odd head-dims]."""
    dim = w.shape[0]
    r = w.reshape(dim, nheads, D // 2, 2)
    return np.concatenate([r[..., 0], r[..., 1]], axis=2).reshape(dim, nheads * D)


def shard_inputs(x, wq, wk, wv, wo, freqs_cos, freqs_sin):
    """Build the 8 per-core input maps (core = 2*block + head_group)."""
    import ml_dtypes
    BF = ml_dtypes.bfloat16
    x = np.ascontiguousarray(np.asarray(x, dtype=np.float32))
    wq_p = _deinterleave_cols(np.asarray(wq, dtype=np.float32), 32)
    wk_p = _deinterleave_cols(np.asarray(wk, dtype=np.float32), 8)
    wv = np.asarray(wv, dtype=np.float32)
    wo = np.asarray(wo, dtype=np.float32)
    cos = np.asarray(freqs_cos, dtype=np.float32)
    sin = np.asarray(freqs_sin, dtype=np.float32)

    wq_h = wq_p.reshape(DIM, 32, D)
    wk_h = wk_p.reshape(DIM, 8, D)
    wv_h = wv.reshape(DIM, 8, D)
    wo_h = wo.reshape(32, D, DIM)

    in_maps = []
    for core in range(N_CORES):
        b, g = divmod(core, 2)
        rows = slice(b * BLOCK, (b + 1) * BLOCK)
        cosT = cos[rows].T                       # [64, block]
        sinT = sin[rows].T
        cos2 = np.concatenate([cosT, cosT], axis=0)     # [128, block]
        sin2 = np.concatenate([-sinT, sinT], axis=0)
        in_maps.append({
            "xbT": np.ascontiguousarray(x[rows, :].T).astype(BF),
            "wq": np.ascontiguousarray(
                wq_h[:, g * HQ:(g + 1) * HQ].reshape(DIM, HQ * D)).astype(BF),
            "wk": np.ascontiguousarray(
                wk_h[:, g * HKV:(g + 1) * HKV].reshape(DIM, HKV * D)).astype(BF),
            "wv": np.ascontiguousarray(
                wv_h[:, g * HKV:(g + 1) * HKV].reshape(DIM, HKV * D)).astype(BF),
            "wo": np.ascontiguousarray(
                wo_h[g * HQ:(g + 1) * HQ].reshape(HQ * D, DIM)).astype(BF),
            "cos2": np.ascontiguousarray(cos2).astype(BF),
            "sin2": np.ascontiguousarray(sin2).astype(BF),
        })
    return in_maps


def unshard_output(core_outs):
    full = np.empty((NB_TOTAL, DIM), dtype=np.float32)
    for b in range(NB_TOTAL // BLOCK):
        full[b * BLOCK:(b + 1) * BLOCK] = \
            np.asarray(core_outs[2 * b], dtype=np.float32) + \
            np.asarray(core_outs[2 * b + 1], dtype=np.float32)
    return full


NB_TOTAL = 4096  # total sequence length

_NC_CACHE = {}


def _get_nc():
    key = (DIM, BLOCK, HQ, HKV)
    if key not in _NC_CACHE:
        _NC_CACHE[key] = build_kernel()
    return _NC_CACHE[key]


def kernel(x, wq, wk, wv, wo, freqs_cos, freqs_sin, block_size, **run_kwargs):
    assert int(block_size) == BLOCK, f"unexpected block_size {block_size}"
    in_maps = shard_inputs(x, wq, wk, wv, wo, freqs_cos, freqs_sin)
    nc = _get_nc()
    res = bass_utils.run_bass_kernel_spmd(
        nc, in_maps, core_ids=list(range(N_CORES)), **run_kwargs)
    outs = [r["out"] for r in res.results]
    out = unshard_output(outs)
    kernel.last_results = res
    return out

